# revision 5
# baseline (speedup 1.0000x reference)
"""Trainium2 Bass kernel for DepthwiseXCorrAug.

Computes, for B=64 samples sharded 8-per-core across 8 NeuronCores:
  k = relu(bn(conv3x3_valid(kernel_in, w_k)))     # [B,256,5,5]
  s = relu(bn(conv3x3_same(search_in, w_s)))      # [B,256,31,31]
  out = per-sample per-channel xcorr(s, k), pad 2 # [B,256,31,31]

Device strategy (per core, all bf16 operands, fp32 PSUM accumulation):
  - conv branches as bf16 matmuls over (ci-block x 3x3-tap) accumulated in
    PSUM; BN folded into weights on host, bias+ReLU applied on PSUM
    eviction (alternating ScalarE / VectorE).
  - depthwise xcorr as bf16 diagonal-weight matmuls on 16 concurrent 32x32
    PE tiles; organized as 2-sample "half-streams" (tile column encodes
    (row-chunk, sample), 4 PSUM banks each).
  - conv is fill-bound (weight bus ~half idle) while the xcorr is
    weight-load-bound (fill path mostly idle), so xcorr taps are WOVEN
    between conv tap-groups: the xcorr weight streams hide under conv
    fills, and wall time approaches total fill time.
"""

import sys

sys.path.insert(0, "/opt/trn_rl_repo")

import numpy as np

import concourse.bass as bass
import concourse.mybir as mybir
import concourse.tile as tile
from concourse import bacc
from concourse.bass_utils import run_bass_kernel_spmd

EPS = 1e-5
N_CORES = 8
B, CIN, HID = 64, 256, 256
SPC = B // N_CORES  # samples per core

_cached_nc = None
last_results = None  # set by kernel(); used by test harness for profiling

CHUNKS = [(0, 16), (16, 15)]  # (y0, nrows) splitting the 31 output rows
WEAVE = 2  # xcorr taps pulled per conv tap-group while weaving


def _build_program():
    f32 = mybir.dt.float32
    bf16 = mybir.dt.bfloat16
    RELU = mybir.ActivationFunctionType.Relu
    ADD = mybir.AluOpType.add
    MAX = mybir.AluOpType.max
    MULT = mybir.AluOpType.mult

    nc = bacc.Bacc("TRN2", target_bir_lowering=False, debug=False,
                   num_devices=N_CORES)

    wTs_d = [nc.dram_tensor(f"wTs{cb}", [128, 2304], bf16, kind="ExternalInput").ap()
             for cb in range(2)]
    wTk_d = [nc.dram_tensor(f"wTk{cb}", [128, 2304], bf16, kind="ExternalInput").ap()
             for cb in range(2)]
    xk_d = [nc.dram_tensor(f"xk{cb}", [128, 1800], bf16, kind="ExternalInput").ap()
            for cb in range(2)]
    xs_d = nc.dram_tensor("xs", [SPC, 2, 128, 33 * 34], bf16, kind="ExternalInput").ap()
    bk_d = nc.dram_tensor("bk", [2, 128, 1], f32, kind="ExternalInput").ap()
    bs_d = nc.dram_tensor("bs", [2, 128, 1], f32, kind="ExternalInput").ap()
    m32_d = nc.dram_tensor("m32", [128, 32], bf16, kind="ExternalInput").ap()
    out_d = nc.dram_tensor("out", [SPC, CIN, 31, 31], f32, kind="ExternalOutput").ap()
    out_flat = out_d.rearrange("s c h w -> s c (h w)")

    with tile.TileContext(nc) as tc:
        with tc.tile_pool(name="wp", bufs=1) as wp, \
             tc.tile_pool(name="xop", bufs=8) as xout_pool, \
             tc.tile_pool(name="ps", bufs=8, space="PSUM") as psp:

            # ---- persistent SBUF tiles ----
            bk = [wp.tile([128, 1], f32, tag=f"bk{ob}", name=f"bk{ob}")
                  for ob in range(2)]
            bs = [wp.tile([128, 1], f32, tag=f"bs{ob}", name=f"bs{ob}")
                  for ob in range(2)]
            m32 = wp.tile([128, 32], bf16, tag="m32", name="m32")
            wTs = [wp.tile([128, 2304], bf16, tag=f"wTs{cb}", name=f"wTs{cb}")
                   for cb in range(2)]
            wTk = [wp.tile([128, 2304], bf16, tag=f"wTk{cb}", name=f"wTk{cb}")
                   for cb in range(2)]
            xk = [wp.tile([128, 1800], bf16, tag=f"xk{cb}", name=f"xk{cb}")
                  for cb in range(2)]
            spin = {}
            for s in range(SPC):
                for cb in range(2):
                    spin[(s, cb)] = wp.tile([128, 33 * 34], bf16,
                                            tag=f"spin{s}_{cb}",
                                            name=f"spin{s}_{cb}")
            spout = {}
            for s in range(SPC):
                for ob in range(2):
                    spout[(s, ob)] = wp.tile([128, 35 * 35], bf16,
                                             tag=f"spout{s}_{ob}",
                                             name=f"spout{s}_{ob}")
            kf = [wp.tile([128, 200], bf16, tag=f"kf{ob}", name=f"kf{ob}")
                  for ob in range(2)]
            # strips: [128, (s, t, c)] per ob; 32x32 block (i, *) of the tap-t
            # weight matrix for sample s is diag(k[s, ob*128+32i : +32, t])
            strips = [wp.tile([128, SPC * 25 * 32], bf16, tag=f"strip{ob}",
                              name=f"strip{ob}") for ob in range(2)]

            # ---- input DMAs in first-needed order ----
            for ob in range(2):
                nc.sync.dma_start(bk[ob][:], bk_d[ob])
                nc.sync.dma_start(bs[ob][:], bs_d[ob])
            nc.sync.dma_start(m32[:], m32_d)
            nc.sync.dma_start(wTs[0][:], wTs_d[0])
            for s in (0, 1):
                nc.sync.dma_start(spin[(s, 0)][:], xs_d[s, 0])
            nc.sync.dma_start(wTs[1][:], wTs_d[1])
            for s in (0, 1):
                nc.sync.dma_start(spin[(s, 1)][:], xs_d[s, 1])
            for cb in range(2):
                nc.sync.dma_start(xk[cb][:], xk_d[cb])
                nc.sync.dma_start(wTk[cb][:], wTk_d[cb])
            for s in range(2, SPC):
                for cb in range(2):
                    nc.sync.dma_start(spin[(s, cb)][:], xs_d[s, cb])

            # ---- spout border zeroing (xcorr reads 2-wide zero halo) ----
            for s in range(SPC):
                for ob in range(2):
                    sov = spout[(s, ob)][:].rearrange("p (h w) -> p h w", h=35, w=35)
                    nc.vector.memset(sov[:, 0:2, :], 0.0)
                    nc.vector.memset(sov[:, 33:35, :], 0.0)
                    nc.vector.memset(sov[:, 2:33, 0:2], 0.0)
                    nc.vector.memset(sov[:, 2:33, 33:35], 0.0)

            # ---- conv_k generator: yields per tap-group ----
            def conv_k_gen():
                for ob in range(2):
                    pk = psp.tile([128, 512], f32, tag="ps", name=f"pk{ob}")
                    idx = 0
                    for cb in range(2):
                        for t in range(9):
                            nc.tensor.matmul(
                                pk[:, 0:200],
                                wTk[cb][:, (t * 2 + ob) * 128:(t * 2 + ob + 1) * 128],
                                xk[cb][:, t * 200:(t + 1) * 200],
                                start=(idx == 0), stop=(idx == 17))
                            idx += 1
                            yield
                    nc.scalar.activation(kf[ob][:], pk[:, 0:200], RELU,
                                         bias=bk[ob][:, 0:1], scale=1.0)
                # strips: one DVE op per ob
                for ob in range(2):
                    out_v = strips[ob][:].rearrange(
                        "p (s t c) -> p s t c", s=SPC, t=25)
                    kf_v = kf[ob][:].rearrange(
                        "p (s t one) -> p s t one", s=SPC,
                        t=25).broadcast_to([128, SPC, 25, 32])
                    m32_v = m32[:].rearrange(
                        "p (o1 o2 c) -> p o1 o2 c", o1=1,
                        o2=1).broadcast_to([128, SPC, 25, 32])
                    nc.vector.tensor_tensor(out_v, kf_v, m32_v, MULT)
                yield

            # ---- conv_s round generator: one (ob, pair), 4 PSUM banks ----
            def conv_s_gen(ob, pair):
                s0 = pair * 2
                ptiles = {}
                for s in (s0, s0 + 1):
                    for ci, (y0, nr) in enumerate(CHUNKS):
                        ptiles[(s, ci)] = psp.tile(
                            [128, 512], f32, tag="ps", name=f"pc{s}_{ob}_{ci}")
                idx = 0
                for cb in range(2):
                    for t in range(9):
                        dy, dx = t // 3, t % 3
                        lhsT = wTs[cb][:, (t * 2 + ob) * 128:(t * 2 + ob + 1) * 128]
                        for s in (s0, s0 + 1):
                            view = spin[(s, cb)][:].rearrange(
                                "p (h w) -> p h w", h=33, w=34)
                            for ci, (y0, nr) in enumerate(CHUNKS):
                                nc.tensor.matmul(
                                    ptiles[(s, ci)][:, 0:nr * 31],
                                    lhsT,
                                    view[:, y0 + dy:y0 + dy + nr, dx:dx + 31],
                                    start=(idx == 0), stop=(idx == 17))
                        idx += 1
                        yield
                n = 0
                for s in (s0, s0 + 1):
                    sov = spout[(s, ob)][:].rearrange(
                        "p (h w) -> p h w", h=35, w=35)
                    for ci, (y0, nr) in enumerate(CHUNKS):
                        pv = ptiles[(s, ci)][:, 0:nr * 31].rearrange(
                            "p (h w) -> p h w", h=nr, w=31)
                        dst = sov[:, 2 + y0:2 + y0 + nr, 2:33]
                        if n % 2 == 0:
                            nc.scalar.activation(dst, pv, RELU,
                                                 bias=bs[ob][:, 0:1], scale=1.0)
                        else:
                            nc.vector.tensor_scalar(dst, pv, bs[ob][:, 0:1],
                                                    0.0, ADD, MAX)
                        n += 1
                yield

            # ---- xcorr half-stream generator: one (pair, ob), 2 samples,
            #      4 PSUM banks; tile column encodes (ci, sample j) ----
            def xcorr_gen(pair, ob):
                s0 = pair * 2
                sovs = [spout[(s0 + j, ob)][:].rearrange(
                    "p (h w) -> p h w", h=35, w=35) for j in range(2)]
                st_v = strips[ob][:].rearrange(
                    "p (s t c) -> p s t c", s=SPC, t=25)
                px = [psp.tile([128, 512], f32, tag="ps",
                               name=f"px{pair}_{ob}_{i}") for i in range(4)]
                for t in range(25):
                    dy, dx = t // 5, t % 5
                    for i in range(4):
                        for ci, (y0, nr) in enumerate(CHUNKS):
                            for j in range(2):
                                nc.tensor.matmul(
                                    px[i][64 * ci + 32 * j:64 * ci + 32 * j + 32,
                                          0:nr * 31],
                                    st_v[32 * i:32 * i + 32, s0 + j, t, :],
                                    sovs[j][32 * i:32 * i + 32,
                                            y0 + dy:y0 + dy + nr, dx:dx + 31],
                                    start=(t == 0), stop=(t == 24),
                                    tile_position=(32 * i, 64 * ci + 32 * j))
                    yield
                n = 0
                for i in range(4):
                    for ci, (y0, nr) in enumerate(CHUNKS):
                        N = nr * 31
                        xo = xout_pool.tile([128, 496], f32, tag="xo",
                                            name=f"xo{pair}_{ob}_{i}_{ci}")
                        src = px[i][64 * ci:64 * ci + 64, 0:N]
                        if n % 2 == 0:
                            nc.vector.tensor_copy(xo[0:64, 0:N], src)
                        else:
                            nc.scalar.copy(xo[0:64, 0:N], src)
                        n += 1
                        dst = out_flat[s0:s0 + 2,
                                       ob * 128 + 32 * i:ob * 128 + 32 * i + 32,
                                       y0 * 31:y0 * 31 + N]
                        nc.sync.dma_start(dst, xo[0:64, 0:N])
                yield

            # ---- the weave ----
            def drain(gen):
                for _ in gen:
                    pass

            def weave(conv_gen, xc_gen, ratio=WEAVE, skip=0):
                """Pull conv tap-groups, interleaving xcorr taps between them.

                `skip` conv groups run unwoven first (lets the woven stream's
                inputs land so its head never blocks the in-order PE queue).
                """
                done_xc = xc_gen is None
                k = 0
                for _ in conv_gen:
                    k += 1
                    if done_xc or k <= skip:
                        continue
                    for _ in range(ratio):
                        if next(xc_gen, "END") == "END":
                            done_xc = True
                            break
                if not done_xc:
                    drain(xc_gen)

            # round order: pair-major so each half-stream is eligible right
            # after its conv pair-round and weaves into the next one
            rounds = [(ob, p) for p in range(4) for ob in range(2)]
            pending = None  # xcorr half-stream awaiting weave
            first = True
            for ob, p in rounds:
                cg = conv_s_gen(ob, p)
                if first:
                    # weave conv_k (+strips) into the first conv round; its
                    # inputs arrive a few microseconds after the conv's, so
                    # lead with unwoven conv groups
                    weave(cg, conv_k_gen(), ratio=3, skip=5)
                    first = False
                else:
                    weave(cg, pending, skip=2)
                pending = xcorr_gen(p, ob)
            drain(pending)  # last half-stream has no conv left to weave into

    nc.compile()
    return nc


def _host_prep(kernel, search, w_k, g_k, b_k, m_k, v_k, w_s, g_s, b_s, m_s, v_s):
    import ml_dtypes
    bf = ml_dtypes.bfloat16

    def fold(w, g, b, m, v):
        scale = g / np.sqrt(v + EPS)
        return (w * scale[:, None, None, None]).astype(np.float32), \
               (b - m * scale).astype(np.float32)

    wkf, bias_k = fold(w_k, g_k, b_k, m_k, v_k)
    wsf, bias_s = fold(w_s, g_s, b_s, m_s, v_s)

    def packT(w):  # [o, ci, 3, 3] -> [cb, ci, (t, ob, o)] bf16
        arr = w.reshape(2, 128, 2, 128, 9).transpose(2, 3, 4, 0, 1)
        return np.ascontiguousarray(arr, dtype=np.float32).astype(bf).reshape(
            2, 128, 2304)

    wTk = packT(wkf)
    wTs = packT(wsf)

    M32 = np.zeros((128, 32), dtype=np.float32)
    for p in range(128):
        M32[p, p % 32] = 1.0
    M32 = M32.astype(bf)

    bk = np.ascontiguousarray(bias_k.reshape(2, 128, 1))
    bs = np.ascontiguousarray(bias_s.reshape(2, 128, 1))

    in_maps = []
    for core in range(N_CORES):
        kin = kernel[core * SPC:(core + 1) * SPC]
        sin = search[core * SPC:(core + 1) * SPC]

        Xk = np.zeros((2, 128, 9, 200), dtype=np.float32)
        for t in range(9):
            dy, dx = t // 3, t % 3
            p = kin[:, :, dy:dy + 5, dx:dx + 5].reshape(SPC, 2, 128, 25)
            Xk[:, :, t, :] = p.transpose(1, 2, 0, 3).reshape(2, 128, 200)
        Xk = Xk.astype(bf).reshape(2, 128, 1800)

        Xs = np.zeros((SPC, 2, 128, 33, 34), dtype=np.float32)
        Xs[:, :, :, 1:32, 1:32] = sin.reshape(SPC, 2, 128, 31, 31)
        Xs = Xs.astype(bf).reshape(SPC, 2, 128, 33 * 34)

        in_maps.append({
            "wTs0": wTs[0], "wTs1": wTs[1],
            "wTk0": wTk[0], "wTk1": wTk[1],
            "xk0": Xk[0], "xk1": Xk[1],
            "xs": Xs, "bk": bk, "bs": bs, "m32": M32,
        })
    return in_maps


def kernel(kernel, search, w_k, g_k, b_k, m_k, v_k, w_s, g_s, b_s, m_s, v_s,
           _trace=False):
    global _cached_nc, last_results
    args = [np.ascontiguousarray(np.asarray(x, dtype=np.float32)) for x in
            (kernel, search, w_k, g_k, b_k, m_k, v_k, w_s, g_s, b_s, m_s, v_s)]
    if _cached_nc is None:
        _cached_nc = _build_program()
    nc = _cached_nc
    in_maps = _host_prep(*args)
    res = run_bass_kernel_spmd(nc, in_maps, core_ids=list(range(N_CORES)),
                               trace=_trace)
    last_results = res
    out = np.concatenate([res.results[i]["out"] for i in range(N_CORES)], axis=0)
    return np.ascontiguousarray(out.astype(np.float32))


# revision 6
# speedup vs baseline: 1.5795x; 1.5795x over previous
"""Trainium2 Bass kernel for DepthwiseXCorrAug.

Computes, for B=64 samples sharded 8-per-core across 8 NeuronCores:
  k = relu(bn(conv3x3_valid(kernel_in, w_k)))     # [B,256,5,5]
  s = relu(bn(conv3x3_same(search_in, w_s)))      # [B,256,31,31]
  out = per-sample per-channel xcorr(s, k), pad 2 # [B,256,31,31]

Device strategy (per core, all bf16 operands, fp32 PSUM accumulation):
  - conv branches as bf16 matmuls over (ci-block x 3x3-tap) accumulated in
    PSUM; BN folded into weights on host, bias+ReLU applied on PSUM
    eviction (alternating ScalarE / VectorE). Fill-bound at ~207ns/MM.
  - depthwise xcorr as bf16 64-diagonal weights on the four 64x64 PE tiles
    (2 samples x 2 channel-halves per tap round); weight-load bound, so
    64-wide tiles halve the instruction count vs 32x32. Row-chunk phases
    (ci) run as separate 4-bank passes so evictions/output DMA overlap the
    next pass and the kernel tail stays short.
"""

import sys

sys.path.insert(0, "/opt/trn_rl_repo")

import numpy as np

import concourse.bass as bass
import concourse.mybir as mybir
import concourse.tile as tile
from concourse import bacc
from concourse.bass_utils import run_bass_kernel_spmd

EPS = 1e-5
N_CORES = 8
B, CIN, HID = 64, 256, 256
SPC = B // N_CORES  # samples per core

_cached_nc = None
last_results = None  # set by kernel(); used by test harness for profiling

CHUNKS = [(0, 16), (16, 15)]  # (y0, nrows) splitting the 31 output rows


def _build_program():
    f32 = mybir.dt.float32
    bf16 = mybir.dt.bfloat16
    RELU = mybir.ActivationFunctionType.Relu
    ADD = mybir.AluOpType.add
    MAX = mybir.AluOpType.max
    MULT = mybir.AluOpType.mult

    nc = bacc.Bacc("TRN2", target_bir_lowering=False, debug=False,
                   num_devices=N_CORES)

    wTs_d = [nc.dram_tensor(f"wTs{cb}", [128, 2304], bf16, kind="ExternalInput").ap()
             for cb in range(2)]
    wTk_d = [nc.dram_tensor(f"wTk{cb}", [128, 2304], bf16, kind="ExternalInput").ap()
             for cb in range(2)]
    xk_d = [nc.dram_tensor(f"xk{cb}", [128, 1800], bf16, kind="ExternalInput").ap()
            for cb in range(2)]
    xs_d = nc.dram_tensor("xs", [SPC, 2, 128, 33 * 34], bf16, kind="ExternalInput").ap()
    bk_d = nc.dram_tensor("bk", [2, 128, 1], f32, kind="ExternalInput").ap()
    bs_d = nc.dram_tensor("bs", [2, 128, 1], f32, kind="ExternalInput").ap()
    m64_d = nc.dram_tensor("m64", [128, 64], bf16, kind="ExternalInput").ap()
    out_d = nc.dram_tensor("out", [SPC, CIN, 31, 31], f32, kind="ExternalOutput").ap()
    out_flat = out_d.rearrange("s c h w -> s c (h w)")

    with tile.TileContext(nc) as tc:
        with tc.tile_pool(name="wp", bufs=1) as wp, \
             tc.tile_pool(name="xop", bufs=8) as xout_pool, \
             tc.tile_pool(name="ps", bufs=8, space="PSUM") as psp:

            # ---- persistent SBUF tiles ----
            bk = [wp.tile([128, 1], f32, tag=f"bk{ob}", name=f"bk{ob}")
                  for ob in range(2)]
            bs = [wp.tile([128, 1], f32, tag=f"bs{ob}", name=f"bs{ob}")
                  for ob in range(2)]
            m64 = wp.tile([128, 64], bf16, tag="m64", name="m64")
            wTs = [wp.tile([128, 2304], bf16, tag=f"wTs{cb}", name=f"wTs{cb}")
                   for cb in range(2)]
            wTk = [wp.tile([128, 2304], bf16, tag=f"wTk{cb}", name=f"wTk{cb}")
                   for cb in range(2)]
            xk = [wp.tile([128, 1800], bf16, tag=f"xk{cb}", name=f"xk{cb}")
                  for cb in range(2)]
            spin = {}
            for s in range(SPC):
                for cb in range(2):
                    spin[(s, cb)] = wp.tile([128, 33 * 34], bf16,
                                            tag=f"spin{s}_{cb}",
                                            name=f"spin{s}_{cb}")
            spout = {}
            for s in range(SPC):
                for ob in range(2):
                    spout[(s, ob)] = wp.tile([128, 35 * 35], bf16,
                                             tag=f"spout{s}_{ob}",
                                             name=f"spout{s}_{ob}")
            kf = [wp.tile([128, 200], bf16, tag=f"kf{ob}", name=f"kf{ob}")
                  for ob in range(2)]
            # strips: [128, (s, t, c64)] per ob; rows 64*RI..64*RI+64 of the
            # tap-t weight for sample s form diag(k[s, ob*128+64*RI : +64, t])
            strips = [wp.tile([128, SPC * 25 * 64], bf16, tag=f"strip{ob}",
                              name=f"strip{ob}") for ob in range(2)]

            # ---- input DMAs in first-needed order ----
            for ob in range(2):
                nc.sync.dma_start(bk[ob][:], bk_d[ob])
                nc.sync.dma_start(bs[ob][:], bs_d[ob])
            nc.sync.dma_start(m64[:], m64_d)
            # first conv taps need only the head of wTs0
            nc.sync.dma_start(wTs[0][:, 0:768], wTs_d[0][:, 0:768])
            for s in (0, 1):
                nc.sync.dma_start(spin[(s, 0)][:], xs_d[s, 0])
            nc.sync.dma_start(wTs[0][:, 768:2304], wTs_d[0][:, 768:2304])
            nc.sync.dma_start(wTs[1][:], wTs_d[1])
            for s in (0, 1):
                nc.sync.dma_start(spin[(s, 1)][:], xs_d[s, 1])
            for cb in range(2):
                nc.sync.dma_start(xk[cb][:], xk_d[cb])
                nc.sync.dma_start(wTk[cb][:], wTk_d[cb])
            for s in range(2, SPC):
                for cb in range(2):
                    nc.sync.dma_start(spin[(s, cb)][:], xs_d[s, cb])

            # ---- spout border zeroing (xcorr reads 2-wide zero halo) ----
            for s in range(SPC):
                for ob in range(2):
                    sov = spout[(s, ob)][:].rearrange("p (h w) -> p h w", h=35, w=35)
                    nc.vector.memset(sov[:, 0:2, :], 0.0)
                    nc.vector.memset(sov[:, 33:35, :], 0.0)
                    nc.vector.memset(sov[:, 2:33, 0:2], 0.0)
                    nc.vector.memset(sov[:, 2:33, 33:35], 0.0)

            # ---- conv_k: all 8 samples batched on the free dim ----
            def emit_conv_k():
                for ob in range(2):
                    pk = psp.tile([128, 512], f32, tag="ps", name=f"pk{ob}")
                    idx = 0
                    for cb in range(2):
                        for t in range(9):
                            nc.tensor.matmul(
                                pk[:, 0:200],
                                wTk[cb][:, (t * 2 + ob) * 128:(t * 2 + ob + 1) * 128],
                                xk[cb][:, t * 200:(t + 1) * 200],
                                start=(idx == 0), stop=(idx == 17))
                            idx += 1
                    nc.scalar.activation(kf[ob][:], pk[:, 0:200], RELU,
                                         bias=bk[ob][:, 0:1], scale=1.0)
                for ob in range(2):
                    out_v = strips[ob][:].rearrange(
                        "p (s t c) -> p s t c", s=SPC, t=25)
                    kf_v = kf[ob][:].rearrange(
                        "p (s t one) -> p s t one", s=SPC,
                        t=25).broadcast_to([128, SPC, 25, 64])
                    m64_v = m64[:].rearrange(
                        "p (o1 o2 c) -> p o1 o2 c", o1=1,
                        o2=1).broadcast_to([128, SPC, 25, 64])
                    nc.vector.tensor_tensor(out_v, kf_v, m64_v, MULT)

            # ---- conv_s round: one (ob, sample-pair), 4 PSUM banks ----
            def conv_s_round(ob, pair):
                s0 = pair * 2
                ptiles = {}
                for s in (s0, s0 + 1):
                    for ci, (y0, nr) in enumerate(CHUNKS):
                        ptiles[(s, ci)] = psp.tile(
                            [128, 512], f32, tag="ps", name=f"pc{s}_{ob}_{ci}")
                idx = 0
                for cb in range(2):
                    for t in range(9):
                        dy, dx = t // 3, t % 3
                        lhsT = wTs[cb][:, (t * 2 + ob) * 128:(t * 2 + ob + 1) * 128]
                        for s in (s0, s0 + 1):
                            view = spin[(s, cb)][:].rearrange(
                                "p (h w) -> p h w", h=33, w=34)
                            for ci, (y0, nr) in enumerate(CHUNKS):
                                nc.tensor.matmul(
                                    ptiles[(s, ci)][:, 0:nr * 31],
                                    lhsT,
                                    view[:, y0 + dy:y0 + dy + nr, dx:dx + 31],
                                    start=(idx == 0), stop=(idx == 17))
                        idx += 1
                n = 0
                for s in (s0, s0 + 1):
                    sov = spout[(s, ob)][:].rearrange(
                        "p (h w) -> p h w", h=35, w=35)
                    for ci, (y0, nr) in enumerate(CHUNKS):
                        pv = ptiles[(s, ci)][:, 0:nr * 31].rearrange(
                            "p (h w) -> p h w", h=nr, w=31)
                        dst = sov[:, 2 + y0:2 + y0 + nr, 2:33]
                        if n % 2 == 0:
                            nc.scalar.activation(dst, pv, RELU,
                                                 bias=bs[ob][:, 0:1], scale=1.0)
                        else:
                            nc.vector.tensor_scalar(dst, pv, bs[ob][:, 0:1],
                                                    0.0, ADD, MAX)
                        n += 1

            # ---- xcorr phase: one (g, ob, ci); 64x64 tiles, 4 PSUM banks.
            # Per sample pair (sA, sB):
            #   bank A: tile (0,0) -> A half0 @ parts 0:64,
            #           tile (1,1) -> A half1 @ parts 64:128
            #   bank B: tile (1,0) -> B half1 @ parts 0:64,
            #           tile (0,1) -> B half0 @ parts 64:128
            def xcorr_phase(g, ob, ci):
                y0, nr = CHUNKS[ci]
                N = nr * 31
                st_v = strips[ob][:].rearrange(
                    "p (s t c) -> p s t c", s=SPC, t=25)
                banks = {}
                for p2 in range(2):
                    for w in ("A", "B"):
                        banks[(p2, w)] = psp.tile(
                            [128, 512], f32, tag="ps",
                            name=f"px{g}_{ob}_{ci}_{p2}{w}")
                for t in range(25):
                    dy, dx = t // 5, t % 5
                    for p2 in range(2):
                        sA = g * 4 + p2 * 2
                        sB = sA + 1
                        va = spout[(sA, ob)][:].rearrange(
                            "p (h w) -> p h w", h=35, w=35)
                        vb = spout[(sB, ob)][:].rearrange(
                            "p (h w) -> p h w", h=35, w=35)
                        bA = banks[(p2, "A")]
                        bB = banks[(p2, "B")]
                        for RI, src, dstb, q in ((0, va, bA, 0), (1, va, bA, 64),
                                                 (1, vb, bB, 0), (0, vb, bB, 64)):
                            nc.tensor.matmul(
                                dstb[q:q + 64, 0:N],
                                st_v[64 * RI:64 * RI + 64,
                                     sA if src is va else sB, t, :],
                                src[64 * RI:64 * RI + 64,
                                    y0 + dy:y0 + dy + nr, dx:dx + 31],
                                start=(t == 0), stop=(t == 24),
                                tile_position=(64 * RI, q))
                n = 0
                for p2 in range(2):
                    sA = g * 4 + p2 * 2
                    sB = sA + 1
                    for w, s in (("A", sA), ("B", sB)):
                        xo = xout_pool.tile([128, 496], f32, tag="xo",
                                            name=f"xo{g}_{ob}_{ci}_{p2}{w}")
                        src = banks[(p2, w)][:, 0:N]
                        if n % 2 == 0:
                            nc.vector.tensor_copy(xo[:, 0:N], src)
                        else:
                            nc.scalar.copy(xo[:, 0:N], src)
                        n += 1
                        if w == "A":
                            dst = out_flat[s, ob * 128:ob * 128 + 128,
                                           y0 * 31:y0 * 31 + N]
                            nc.sync.dma_start(dst, xo[:, 0:N])
                        else:
                            # bank B holds half1 at parts 0:64, half0 at 64:128
                            nc.sync.dma_start(
                                out_flat[s, ob * 128 + 64:ob * 128 + 128,
                                         y0 * 31:y0 * 31 + N], xo[0:64, 0:N])
                            nc.sync.dma_start(
                                out_flat[s, ob * 128:ob * 128 + 64,
                                         y0 * 31:y0 * 31 + N], xo[64:128, 0:N])

            # ---- schedule ----
            conv_s_round(0, 0)
            conv_s_round(0, 1)
            emit_conv_k()
            conv_s_round(1, 0)
            conv_s_round(1, 1)
            xcorr_phase(0, 0, 0)
            xcorr_phase(0, 0, 1)
            conv_s_round(0, 2)
            xcorr_phase(0, 1, 0)
            xcorr_phase(0, 1, 1)
            conv_s_round(0, 3)
            conv_s_round(1, 2)
            xcorr_phase(1, 0, 0)
            xcorr_phase(1, 0, 1)
            conv_s_round(1, 3)
            xcorr_phase(1, 1, 0)
            xcorr_phase(1, 1, 1)

    nc.compile()
    return nc


def _host_prep(kernel, search, w_k, g_k, b_k, m_k, v_k, w_s, g_s, b_s, m_s, v_s):
    import ml_dtypes
    bf = ml_dtypes.bfloat16

    def fold(w, g, b, m, v):
        scale = g / np.sqrt(v + EPS)
        return (w * scale[:, None, None, None]).astype(np.float32), \
               (b - m * scale).astype(np.float32)

    wkf, bias_k = fold(w_k, g_k, b_k, m_k, v_k)
    wsf, bias_s = fold(w_s, g_s, b_s, m_s, v_s)

    def packT(w):  # [o, ci, 3, 3] -> [cb, ci, (t, ob, o)] bf16
        arr = w.reshape(2, 128, 2, 128, 9).transpose(2, 3, 4, 0, 1)
        return np.ascontiguousarray(arr, dtype=np.float32).astype(bf).reshape(
            2, 128, 2304)

    wTk = packT(wkf)
    wTs = packT(wsf)

    M64 = np.zeros((128, 64), dtype=np.float32)
    for p in range(128):
        M64[p, p % 64] = 1.0
    M64 = M64.astype(bf)

    bk = np.ascontiguousarray(bias_k.reshape(2, 128, 1))
    bs = np.ascontiguousarray(bias_s.reshape(2, 128, 1))

    in_maps = []
    for core in range(N_CORES):
        kin = kernel[core * SPC:(core + 1) * SPC]
        sin = search[core * SPC:(core + 1) * SPC]

        Xk = np.zeros((2, 128, 9, 200), dtype=np.float32)
        for t in range(9):
            dy, dx = t // 3, t % 3
            p = kin[:, :, dy:dy + 5, dx:dx + 5].reshape(SPC, 2, 128, 25)
            Xk[:, :, t, :] = p.transpose(1, 2, 0, 3).reshape(2, 128, 200)
        Xk = Xk.astype(bf).reshape(2, 128, 1800)

        Xs = np.zeros((SPC, 2, 128, 33, 34), dtype=np.float32)
        Xs[:, :, :, 1:32, 1:32] = sin.reshape(SPC, 2, 128, 31, 31)
        Xs = Xs.astype(bf).reshape(SPC, 2, 128, 33 * 34)

        in_maps.append({
            "wTs0": wTs[0], "wTs1": wTs[1],
            "wTk0": wTk[0], "wTk1": wTk[1],
            "xk0": Xk[0], "xk1": Xk[1],
            "xs": Xs, "bk": bk, "bs": bs, "m64": M64,
        })
    return in_maps


def kernel(kernel, search, w_k, g_k, b_k, m_k, v_k, w_s, g_s, b_s, m_s, v_s,
           _trace=False):
    global _cached_nc, last_results
    args = [np.ascontiguousarray(np.asarray(x, dtype=np.float32)) for x in
            (kernel, search, w_k, g_k, b_k, m_k, v_k, w_s, g_s, b_s, m_s, v_s)]
    if _cached_nc is None:
        _cached_nc = _build_program()
    nc = _cached_nc
    in_maps = _host_prep(*args)
    res = run_bass_kernel_spmd(nc, in_maps, core_ids=list(range(N_CORES)),
                               trace=_trace)
    last_results = res
    out = np.concatenate([res.results[i]["out"] for i in range(N_CORES)], axis=0)
    return np.ascontiguousarray(out.astype(np.float32))
